# revision 4
# baseline (speedup 1.0000x reference)
"""Trainium2 Bass kernel for nn_CustomLoss: sum((predicted - target)**2) / 2.

Data-parallel across 8 NeuronCores: rows are sharded, each core streams its
128 MiB shard through SBUF and computes per-partition partial sums of
squared differences; the host sums the 8x128xNT partials and halves.

pred/target are interleaved host-side into one DRAM tensor [NT, P, 2, F] so
each tile is a single 8 MiB DMA with contiguous 64 KiB per partition.

Raw Bass (not Tile): the walrus codegen on this path allows only one sync
wait per compute instruction, so sync is explicit standalone wait_ge's in a
classic double-buffered SP(DMA) -> DVE(sub, mul+reduce) pipeline.

Self-contained: hardcodes shapes from the problem spec; only depends on the
container's bass/concourse install at /opt/trn_rl_repo.
"""

import sys

if "/opt/trn_rl_repo" not in sys.path:
    sys.path.insert(0, "/opt/trn_rl_repo")

import numpy as np

N, D = 1048576, 128
NCORES = 8
ELEMS_PER_CORE = (N // NCORES) * D  # 16,777,216 fp32 = 64 MiB per tensor
P = 128          # SBUF partitions
F = 8192         # free elems per tile per tensor -> 8 MiB DMA per tile
NT = ELEMS_PER_CORE // (P * F)  # 16 tiles
NSLOTS = 2       # double buffering

# Set by test harness to capture a HW profile; harness-default is plain run.
TRACE = False
LAST_EXEC_NS = None

_cached_nc = None


def _build():
    from concourse import bass, mybir

    nc = bass.Bass()
    f32 = mybir.dt.float32
    data_ext = nc.declare_dram_parameter("data", [NT, P, 2, F], f32, isOutput=False)
    out_ext = nc.declare_dram_parameter("partials", [P, NT], f32, isOutput=True)

    with (
        nc.semaphore("dma_a") as dma_a,
        nc.semaphore("dma_b") as dma_b,
        nc.semaphore("dve_sem") as dve_sem,
        nc.semaphore("act_sem") as act_sem,
        nc.semaphore("out_sem") as out_sem,
        nc.sbuf_tensor("tile_a", [P, 2, F], f32) as tile_a,
        nc.sbuf_tensor("tile_b", [P, 2, F], f32) as tile_b,
        nc.sbuf_tensor("acc", [P, NT], f32) as acc,
        nc.Block() as block,
    ):
        tiles = [tile_a, tile_b]
        dsems = [dma_a, dma_b]

        @block.sync
        def _(sync):
            for i in range(NT):
                if i >= NSLOTS:
                    # slot reused: ACT must be done with tile i - NSLOTS
                    sync.wait_ge(act_sem, i - NSLOTS + 1)
                sync.dma_start(
                    out=tiles[i % NSLOTS][:], in_=data_ext[i]
                ).then_inc(dsems[i % NSLOTS], 16)
            sync.wait_ge(act_sem, NT)
            sync.dma_start(out=out_ext[:], in_=acc[:]).then_inc(out_sem, 16)
            sync.wait_ge(out_sem, 16)

        @block.vector
        def _(vector):
            for i in range(NT):
                s = i % NSLOTS
                vector.wait_ge(dsems[s], 16 * (i // NSLOTS + 1))
                t = tiles[s]
                # diff in place over the pred half
                vector.tensor_sub(out=t[:, 0], in0=t[:, 0], in1=t[:, 1]).then_inc(
                    dve_sem, 1
                )

        @block.scalar
        def _(scalar):
            for i in range(NT):
                s = i % NSLOTS
                scalar.wait_ge(dve_sem, i + 1)
                t = tiles[s]
                # square (scratch into the target half) + per-partition sum
                scalar.activation(
                    out=t[:, 1],
                    in_=t[:, 0],
                    func=mybir.ActivationFunctionType.Square,
                    accum_out=acc[:, i : i + 1],
                ).then_inc(act_sem, 1)

    return nc


def kernel(predicted, target):
    global _cached_nc, LAST_EXEC_NS
    from concourse.bass_utils import run_bass_kernel_spmd

    if _cached_nc is None:
        _cached_nc = _build()
    nc = _cached_nc

    p = np.asarray(predicted, dtype=np.float32).reshape(NCORES, NT, P, F)
    t = np.asarray(target, dtype=np.float32).reshape(NCORES, NT, P, F)
    data = np.empty((NCORES, NT, P, 2, F), dtype=np.float32)
    data[:, :, :, 0, :] = p
    data[:, :, :, 1, :] = t
    in_maps = [{"data": data[c]} for c in range(NCORES)]
    res = run_bass_kernel_spmd(nc, in_maps, list(range(NCORES)), trace=TRACE)
    LAST_EXEC_NS = res.exec_time_ns
    total = sum(r["partials"].sum(dtype=np.float64) for r in res.results)
    return np.float32(total / 2.0)


# revision 7
# speedup vs baseline: 1.0837x; 1.0837x over previous
"""Trainium2 Bass kernel for nn_CustomLoss: sum((predicted - target)**2) / 2.

Data-parallel across 8 NeuronCores: rows are sharded, each core streams its
128 MiB shard through SBUF and computes per-partition partial sums of
squared differences; the host sums the 8x128xNSEQ partials and halves.

Raw Bass (not Tile): the walrus codegen on this path allows only one sync
wait per compute instruction, so sync is explicit standalone wait_ge's.

Pipeline per core, double-buffered (2 slots per stream):
  SP ring   : pred tile DMAs (HWDGE queue 1)
  ACT ring  : targ tile DMAs (HWDGE queue 2, interleaved with squares)
  DVE       : diff = pred - targ (in place)
  ACT       : square(diff) + per-partition accumulate -> acc[:, seq]
The last full tile is split into 4 quarter chunks so the compute tail after
the final DMA byte is ~4 us instead of ~16 us.

Self-contained: hardcodes shapes from the problem spec; only depends on the
container's bass/concourse install at /opt/trn_rl_repo.
"""

import sys

if "/opt/trn_rl_repo" not in sys.path:
    sys.path.insert(0, "/opt/trn_rl_repo")

import numpy as np

N, D = 1048576, 128
NCORES = 8
ELEMS_PER_CORE = (N // NCORES) * D  # 16,777,216 fp32 = 64 MiB per tensor
P = 128                    # SBUF partitions
FTOT = ELEMS_PER_CORE // P  # 131072 fp32 per partition per tensor
FBIG = 8192                # full tile: 4 MiB per tensor per DMA
FSMALL = 2048              # tail chunks: 1 MiB per tensor per DMA
NFULL = 15
NSMALL = 4
assert NFULL * FBIG + NSMALL * FSMALL == FTOT
# (column offset, width) per pipeline iteration
CHUNKS = [(i * FBIG, FBIG) for i in range(NFULL)] + [
    (NFULL * FBIG + k * FSMALL, FSMALL) for k in range(NSMALL)
]
NSEQ = len(CHUNKS)

# Set by test harness to capture a HW profile; harness-default is plain run.
TRACE = False
LAST_EXEC_NS = None

_cached_nc = None


def _build():
    from concourse import bass, mybir

    nc = bass.Bass()
    f32 = mybir.dt.float32
    pred_ext = nc.declare_dram_parameter("predicted", [P, FTOT], f32, isOutput=False)
    targ_ext = nc.declare_dram_parameter("target", [P, FTOT], f32, isOutput=False)
    out_ext = nc.declare_dram_parameter("partials", [P, NSEQ], f32, isOutput=True)

    with (
        nc.semaphore("psem_a") as psem_a,
        nc.semaphore("psem_b") as psem_b,
        nc.semaphore("tsem_a") as tsem_a,
        nc.semaphore("tsem_b") as tsem_b,
        nc.semaphore("dve_sem") as dve_sem,
        nc.semaphore("act_sem") as act_sem,
        nc.semaphore("out_sem") as out_sem,
        nc.sbuf_tensor("pred_a", [P, FBIG], f32) as pred_a,
        nc.sbuf_tensor("pred_b", [P, FBIG], f32) as pred_b,
        nc.sbuf_tensor("targ_a", [P, FBIG], f32) as targ_a,
        nc.sbuf_tensor("targ_b", [P, FBIG], f32) as targ_b,
        nc.sbuf_tensor("scratch", [P, FBIG], f32) as scratch,
        nc.sbuf_tensor("acc", [P, NSEQ], f32) as acc,
        nc.Block() as block,
    ):
        pred_t = [pred_a, pred_b]
        targ_t = [targ_a, targ_b]
        psem = [psem_a, psem_b]
        tsem = [tsem_a, tsem_b]

        @block.sync
        def _(sync):
            nslot = [0, 0]
            for seq, (off, w) in enumerate(CHUNKS):
                s = seq % 2
                nslot[s] += 1
                if seq >= 2:
                    # slot reused: ACT must be done reading pred tile seq-2
                    sync.wait_ge(act_sem, seq - 1)
                sync.dma_start(
                    out=pred_t[s][:, 0:w], in_=pred_ext[:, off : off + w]
                ).then_inc(psem[s], 16)
            sync.wait_ge(act_sem, NSEQ)
            sync.dma_start(out=out_ext[:], in_=acc[:]).then_inc(out_sem, 16)
            sync.wait_ge(out_sem, 16)

        @block.vector
        def _(vector):
            nslot = [0, 0]
            for seq, (off, w) in enumerate(CHUNKS):
                s = seq % 2
                nslot[s] += 1
                vector.wait_ge(psem[s], 16 * nslot[s])
                vector.wait_ge(tsem[s], 16 * nslot[s])
                # diff in place over the pred tile
                vector.tensor_sub(
                    out=pred_t[s][:, 0:w],
                    in0=pred_t[s][:, 0:w],
                    in1=targ_t[s][:, 0:w],
                ).then_inc(dve_sem, 1)

        @block.scalar
        def _(scalar):
            # targ tile DMAs ride the ACT HWDGE ring, interleaved with the
            # squares; slot-reuse safety is ACT program order (the square of
            # seq-2 precedes the trigger for seq).
            nslot = [0, 0]
            for seq in range(min(2, NSEQ)):
                s = seq % 2
                nslot[s] += 1
                off, w = CHUNKS[seq]
                scalar.dma_start(
                    out=targ_t[s][:, 0:w], in_=targ_ext[:, off : off + w]
                ).then_inc(tsem[s], 16)
            for seq, (off, w) in enumerate(CHUNKS):
                s = seq % 2
                scalar.wait_ge(dve_sem, seq + 1)
                # square(diff) + row-sum; out goes to a dedicated scratch
                # tile (never a DMA target: the ACT engine's trailing SBUF
                # writes are NOT ordered against a DMA triggered right
                # after on the same queue, so writing into targ_t here
                # would clobber the next targ DMA's data)
                scalar.activation(
                    out=scratch[:, 0:w],
                    in_=pred_t[s][:, 0:w],
                    func=mybir.ActivationFunctionType.Square,
                    accum_out=acc[:, seq : seq + 1],
                ).then_inc(act_sem, 1)
                nxt = seq + 2
                if nxt < NSEQ:
                    noff, nw = CHUNKS[nxt]
                    nslot[s] += 1
                    scalar.dma_start(
                        out=targ_t[s][:, 0:nw], in_=targ_ext[:, noff : noff + nw]
                    ).then_inc(tsem[s], 16)

    return nc


def kernel(predicted, target):
    global _cached_nc, LAST_EXEC_NS
    from concourse.bass_utils import run_bass_kernel_spmd

    if _cached_nc is None:
        _cached_nc = _build()
    nc = _cached_nc

    p = np.ascontiguousarray(np.asarray(predicted, dtype=np.float32)).reshape(
        NCORES, P, FTOT
    )
    t = np.ascontiguousarray(np.asarray(target, dtype=np.float32)).reshape(
        NCORES, P, FTOT
    )
    in_maps = [{"predicted": p[c], "target": t[c]} for c in range(NCORES)]
    res = run_bass_kernel_spmd(nc, in_maps, list(range(NCORES)), trace=TRACE)
    LAST_EXEC_NS = res.exec_time_ns
    total = sum(r["partials"].sum(dtype=np.float64) for r in res.results)
    return np.float32(total / 2.0)


# revision 12
# speedup vs baseline: 1.2368x; 1.1413x over previous
"""Trainium2 Bass kernel for nn_CustomLoss: sum((predicted - target)**2) / 2.

Data-parallel across 8 NeuronCores: rows are sharded, each core streams its
128 MiB shard through SBUF and computes per-partition partial sums of
squared differences; the host sums the 8x128xNSEQ partials and halves.

Raw Bass (not Tile): the walrus codegen on this path allows only one sync
wait per compute instruction, so sync is explicit standalone wait_ge's.

Pipeline per core, double-buffered (2 slots per stream):
  SP ring   : pred tile DMAs (HWDGE queue 1)
  ACT ring  : targ tile DMAs (HWDGE queue 2, interleaved with squares)
  DVE       : diff = pred - targ (in place)
  ACT       : square(diff) + per-partition accumulate -> acc[:, seq]

Tail: the last 16 MiB is split into 8 quarter-width chunks with dedicated
column regions inside the two slot buffers, so every tail DMA is in flight
as soon as the last two full-tile squares retire and the post-last-byte
tail is ~5 us instead of ~16 us.

The Bass-init all-engine barrier is suppressed: its only purpose is
ordering the Pool const-AP memsets against consumers, and this kernel uses
an ACT-local memzero'd bias tile instead of the const APs.

Self-contained: hardcodes shapes from the problem spec; only depends on the
container's bass/concourse install at /opt/trn_rl_repo.
"""

import sys

if "/opt/trn_rl_repo" not in sys.path:
    sys.path.insert(0, "/opt/trn_rl_repo")

import numpy as np

N, D = 1048576, 128
NCORES = 8
ELEMS_PER_CORE = (N // NCORES) * D  # 16,777,216 fp32 = 64 MiB per tensor
P = 128                    # SBUF partitions
FTOT = ELEMS_PER_CORE // P  # 131072 fp32 per partition per tensor
FBIG = 8192                # full tile: 4 MiB per tensor per DMA
FSMALL = 2048              # tail chunks: 1 MiB per tensor per DMA
NFULL = 14
NCHUNK = 8
assert NFULL * FBIG + NCHUNK * FSMALL == FTOT

# Per pipeline iteration: (dram col offset, width, slot, slot col offset).
# Full tiles alternate slots; tail chunks get dedicated quarter regions --
# slot 0 chunks first (they only need the last slot-0 full square, seq 12),
# then slot 1 chunks (gated on seq 13).
SEQS = [(i * FBIG, FBIG, i % 2, 0) for i in range(NFULL)]
for j in range(NCHUNK):
    slot = 0 if j < 4 else 1
    col = (j % 4) * FSMALL
    SEQS.append((NFULL * FBIG + j * FSMALL, FSMALL, slot, col))
NSEQ = len(SEQS)
# act_sem value required before a chunk's pred DMA may overwrite its slot
CHUNK_GATE = {0: NFULL - 1, 1: NFULL}  # ACT of seq 12 / seq 13 done

# Set by test harness to capture a HW profile; harness-default is plain run.
TRACE = False
LAST_EXEC_NS = None

_cached_nc = None


def _build():
    from concourse import bass, mybir

    # Suppress the Bass-init all-engine barrier (see module docstring).
    orig_barrier = bass.Bass.all_engine_barrier
    bass.Bass.all_engine_barrier = lambda self, *a, **k: None
    try:
        nc = bass.Bass()
    finally:
        bass.Bass.all_engine_barrier = orig_barrier

    f32 = mybir.dt.float32
    pred_ext = nc.declare_dram_parameter("predicted", [P, FTOT], f32, isOutput=False)
    targ_ext = nc.declare_dram_parameter("target", [P, FTOT], f32, isOutput=False)
    out_ext = nc.declare_dram_parameter("partials", [P, NSEQ], f32, isOutput=True)

    from contextlib import ExitStack

    ctx = ExitStack()
    # one sem per chunk DMA: several chunk DMAs to the same slot are in
    # flight at once, so a shared counting sem would be unsound (sem total
    # can reach the target while one transfer is still partial)
    cp_p = [ctx.enter_context(nc.semaphore(f"cp_p{j}")) for j in range(NCHUNK)]
    cp_t = [ctx.enter_context(nc.semaphore(f"cp_t{j}")) for j in range(NCHUNK)]

    with (
        ctx,
        nc.semaphore("psem_a") as psem_a,
        nc.semaphore("psem_b") as psem_b,
        nc.semaphore("tsem_a") as tsem_a,
        nc.semaphore("tsem_b") as tsem_b,
        nc.semaphore("dve_sem") as dve_sem,
        nc.semaphore("act_sem") as act_sem,
        nc.semaphore("out_sem") as out_sem,
        nc.sbuf_tensor("pred_a", [P, FBIG], f32) as pred_a,
        nc.sbuf_tensor("pred_b", [P, FBIG], f32) as pred_b,
        nc.sbuf_tensor("targ_a", [P, FBIG], f32) as targ_a,
        nc.sbuf_tensor("targ_b", [P, FBIG], f32) as targ_b,
        nc.sbuf_tensor("scratch", [P, FBIG], f32) as scratch,
        nc.sbuf_tensor("zbias", [P, 1], f32) as zbias,
        nc.sbuf_tensor("acc", [P, NSEQ], f32) as acc,
        nc.Block() as block,
    ):
        pred_t = [pred_a, pred_b]
        targ_t = [targ_a, targ_b]
        psem = [psem_a, psem_b]
        tsem = [tsem_a, tsem_b]

        def targ_dma(eng, seq):
            off, w, s, col = SEQS[seq]
            sem = tsem[s] if w == FBIG else cp_t[seq - NFULL]
            eng.dma_start(
                out=targ_t[s][:, col : col + w], in_=targ_ext[:, off : off + w]
            ).then_inc(sem, 16)

        @block.sync
        def _(sync):
            last_wait = -1
            for seq, (off, w, s, col) in enumerate(SEQS):
                if seq >= 2:
                    # slot (region) reused: the square reading its previous
                    # tenant must be done
                    gate = seq - 1 if w == FBIG else CHUNK_GATE[s]
                    if gate > last_wait:
                        sync.wait_ge(act_sem, gate)
                        last_wait = gate
                sync.dma_start(
                    out=pred_t[s][:, col : col + w], in_=pred_ext[:, off : off + w]
                ).then_inc(psem[s] if w == FBIG else cp_p[seq - NFULL], 16)
            sync.wait_ge(act_sem, NSEQ)
            sync.dma_start(out=out_ext[:], in_=acc[:]).then_inc(out_sem, 16)
            sync.wait_ge(out_sem, 16)

        @block.vector
        def _(vector):
            nslot = [0, 0]
            for seq, (off, w, s, col) in enumerate(SEQS):
                if w == FBIG:
                    nslot[s] += 1
                    vector.wait_ge(psem[s], 16 * nslot[s])
                    vector.wait_ge(tsem[s], 16 * nslot[s])
                else:
                    vector.wait_ge(cp_p[seq - NFULL], 16)
                    vector.wait_ge(cp_t[seq - NFULL], 16)
                # diff in place over the pred tile region
                vector.tensor_sub(
                    out=pred_t[s][:, col : col + w],
                    in0=pred_t[s][:, col : col + w],
                    in1=targ_t[s][:, col : col + w],
                ).then_inc(dve_sem, 1)

        @block.scalar
        def _(scalar):
            # zero bias for Square, owned by ACT itself (program order makes
            # it visible to every square; avoids the framework const APs and
            # therefore any dependence on the suppressed init barrier)
            scalar.memzero(zbias[:])
            # targ tile DMAs ride the ACT HWDGE ring, interleaved with the
            # squares; slot-reuse safety is ACT program order (the square of
            # the previous tenant precedes each trigger).
            targ_dma(scalar, 0)
            targ_dma(scalar, 1)
            for seq, (off, w, s, col) in enumerate(SEQS):
                scalar.wait_ge(dve_sem, seq + 1)
                # square(diff) + row-sum; out goes to a dedicated scratch
                # tile (never a DMA target: the ACT engine's trailing SBUF
                # writes are NOT ordered against a DMA triggered right after
                # on the same queue)
                scalar.activation(
                    out=scratch[:, 0:w],
                    in_=pred_t[s][:, col : col + w],
                    func=mybir.ActivationFunctionType.Square,
                    bias=zbias[:],
                    accum_out=acc[:, seq : seq + 1],
                ).then_inc(act_sem, 1)
                if seq + 2 < NFULL:
                    targ_dma(scalar, seq + 2)
                elif seq == NFULL - 2:  # seq 12: slot-0 chunks now safe
                    for j in range(NFULL, NFULL + 4):
                        targ_dma(scalar, j)
                elif seq == NFULL - 1:  # seq 13: slot-1 chunks now safe
                    for j in range(NFULL + 4, NSEQ):
                        targ_dma(scalar, j)

    return nc


def kernel(predicted, target):
    global _cached_nc, LAST_EXEC_NS
    from concourse.bass_utils import run_bass_kernel_spmd

    if _cached_nc is None:
        _cached_nc = _build()
    nc = _cached_nc

    p = np.ascontiguousarray(np.asarray(predicted, dtype=np.float32)).reshape(
        NCORES, P, FTOT
    )
    t = np.ascontiguousarray(np.asarray(target, dtype=np.float32)).reshape(
        NCORES, P, FTOT
    )
    in_maps = [{"predicted": p[c], "target": t[c]} for c in range(NCORES)]
    res = run_bass_kernel_spmd(nc, in_maps, list(range(NCORES)), trace=TRACE)
    LAST_EXEC_NS = res.exec_time_ns
    total = sum(r["partials"].sum(dtype=np.float64) for r in res.results)
    return np.float32(total / 2.0)
